# revision 9
# baseline (speedup 1.0000x reference)
"""AnyPrecisionLinear (4-bit LUT dequant + CSR outliers + bias) on 8 TRN2 cores.

Sharding: 4-way over out_features (O) x 2-way over tokens (B*S).
Core c handles o in [1024*(c%4), +1024), tokens [4096*(c//4), +4096).

Device does all value math:
  - W (bf16) built from lut via GPSIMD local_scatter with host-computed slot
    tables (pure index preprocessing of qweight bits): pattern = full 16-entry
    lut repeated, scattered to every position, so W[o,i] = lut[o, idx[o,i]].
  - CSR outlier values cast + scattered on device into a small tile, added to W.
  - x is laid out i-major on host (pure transpose/reshape), converted
    f32->bf16 on ScalarE on device; no on-chip x transposes needed.
  - W transposed to i-major on the PE (identity transpose), drained by ScalarE.
  - GEMM on TensorE (bf16, f32 PSUM accum); bias added in the DVE drain.
Host does only layout/index work: sharding, transpose/reshape, bit-plane ->
index repack, sort/slot tables, CSR indptr parsing + dedup, output concat.
"""

import numpy as np
from contextlib import ExitStack

# Problem constants (hardcoded per harness contract).
B, S, I, O = 4, 2048, 4096, 4096
W_BITS = 4
NT_TOTAL = B * S          # 8192 tokens
N_CORES = 8
O_WAY, N_WAY = 4, 2       # sharding grid
O_SH = O // O_WAY         # 1024 out features per core
N_SH = NT_TOTAL // N_WAY  # 4096 tokens per core
NCHUNK = N_SH // 128      # 32 token chunks per core
OT = O_SH // 128          # 8 o-tiles of 128 rows per core
CH = 1024                 # i-chunk size for local_scatter (num_elems limit 2046)
NCH = I // CH             # 4
IC = I // 128             # 32 i-blocks for the GEMM
G = 512                   # tokens per x stage-in group
NG = N_SH // G            # 8
CPG = G // 128            # 4 token chunks per group

_GRAPH_CACHE = {}

import ml_dtypes

_EYE = np.eye(128, dtype=ml_dtypes.bfloat16)


def _host_indices(qweight):
    """bit-planes -> 4-bit index array [O, I] (uint8). Pure bit relayout."""
    shifts = np.arange(32, dtype=np.int32)
    # bits[b, o, w, s] = bit s of qweight[b, o, w]
    bits = ((qweight[:, :, :, None] >> shifts) & 1).astype(np.uint8)
    planew = (1 << (W_BITS - 1 - np.arange(W_BITS))).astype(np.uint8)
    idx = (bits * planew[:, None, None, None]).sum(axis=0, dtype=np.int32)
    return idx.reshape(O, I).astype(np.uint8)


def _scatter_tables(idx, rows, cols, vals):
    """Slot tables for the two device scatters (pure index preprocessing).

    Dequant scatter: slot 16*r+v holds the position of the r-th occurrence of
    lut-code v within the chunk, so scattering the repeated 16-entry lut
    pattern writes lut[o, idx[o,i]] at every position. CSR scatter: slot j of
    chunk ch holds the position of the j-th outlier; its (deduped) value is
    scattered into a zeroed tile and added on top.

    Returns:
      tbl    [O, NCH, NI]  int16  dequant scatter indices (-1 pad)
      ctb    [O, NCH, CSE] int16  csr scatter indices (-1 pad)
      cvals  [O, NCH, CSE] f32    csr value of each slot (0 pad)
      NI, CSE
    """
    # ---- CSR -> dedup'd COO ----
    nnz = cols.shape[0]
    row_ids = (np.searchsorted(rows, np.arange(nnz), side="right") - 1).astype(np.int64)
    key = row_ids * I + cols.astype(np.int64)
    uk, inv = np.unique(key, return_inverse=True)
    v2 = np.zeros(len(uk), np.float64)
    np.add.at(v2, inv, vals.astype(np.float64))
    r2 = uk // I
    c2 = uk % I
    ch2 = c2 // CH
    cl2 = (c2 % CH).astype(np.int16)
    grp = r2 * NCH + ch2  # ascending (uk sorted)
    _, gstart, gcount = np.unique(grp, return_index=True, return_counts=True)
    CSE = max(int(gcount.max()), 2)
    CSE += CSE % 2
    rank = np.arange(len(uk)) - np.repeat(gstart, gcount)
    ctb = np.full((O, NCH, CSE), -1, np.int16)
    ctb[r2, ch2, rank] = cl2
    cvals = np.zeros((O, NCH, CSE), np.float32)
    cvals[r2, ch2, rank] = v2.astype(np.float32)

    # ---- dequant slots (all 16 codes, csr positions included) ----
    idx4 = idx.reshape(O, NCH, CH).astype(np.int16)
    order = np.argsort(idx4, axis=-1, kind="stable").astype(np.int16)
    sortedv = np.take_along_axis(idx4, order.astype(np.int64), axis=-1)
    cnt = np.zeros((O, NCH, 16), np.int32)
    for v in range(16):
        cnt[:, :, v] = (idx4 == v).sum(-1)
    S = int(cnt.max())
    NI = 16 * S
    cstart = np.concatenate(
        [np.zeros((O, NCH, 1), np.int32), np.cumsum(cnt, -1)[:, :, :-1]], -1
    )
    srank = np.arange(CH)[None, None, :] - np.take_along_axis(
        cstart, sortedv.astype(np.int64), axis=-1
    )
    tbl = np.full((O, NCH, NI), -1, np.int16)
    flat = (16 * srank + sortedv).astype(np.int64)
    np.put_along_axis(tbl, flat, order, axis=-1)
    return tbl, ctb, cvals, NI, CSE


def _build_graph(NI, CS):
    import concourse.bass as bass
    import concourse.bacc as bacc
    import concourse.tile as tile
    from concourse import mybir

    dt = mybir.dt
    nc = bacc.Bacc("TRN2", target_bir_lowering=False, debug=False)

    CSE = CS
    xt_d = nc.dram_tensor("xt", [IC, 128, N_SH], dt.float32, kind="ExternalInput")
    lut_d = nc.dram_tensor("lut", [OT, 128, 16], dt.float32, kind="ExternalInput")
    qid_d = nc.dram_tensor("qidx", [OT, 128, NCH * NI], dt.int16, kind="ExternalInput")
    ctb_d = nc.dram_tensor("ctb", [OT, 128, NCH * CSE], dt.int16, kind="ExternalInput")
    cva_d = nc.dram_tensor("cvals", [OT, 128, NCH * CSE], dt.float32, kind="ExternalInput")
    bias_d = nc.dram_tensor("bias", [1, O_SH], dt.float32, kind="ExternalInput")
    eye_d = nc.dram_tensor("eye", [128, 128], dt.bfloat16, kind="ExternalInput")
    y_d = nc.dram_tensor("y", [NCHUNK, 128, O_SH], dt.float32, kind="ExternalOutput")

    with tile.TileContext(nc) as tc, ExitStack() as ctx:
        const = ctx.enter_context(tc.tile_pool(name="const", bufs=1))
        wpool = ctx.enter_context(tc.tile_pool(name="w", bufs=2))
        spool = ctx.enter_context(tc.tile_pool(name="scat", bufs=2))
        qpool = ctx.enter_context(tc.tile_pool(name="qp", bufs=2))
        mpool = ctx.enter_context(tc.tile_pool(name="mp", bufs=1))
        xfpool = ctx.enter_context(tc.tile_pool(name="xf", bufs=3))
        xgpool = ctx.enter_context(tc.tile_pool(name="xg", bufs=2))
        yopool = ctx.enter_context(tc.tile_pool(name="yo", bufs=2))
        psum = ctx.enter_context(
            tc.tile_pool(name="ps", bufs=4, space=bass.MemorySpace.PSUM)
        )
        pst = ctx.enter_context(
            tc.tile_pool(name="pst", bufs=2, space=bass.MemorySpace.PSUM)
        )
        psb = ctx.enter_context(
            tc.tile_pool(name="psb", bufs=1, space=bass.MemorySpace.PSUM)
        )

        # Resident transposed weights: WT[p, 1024*ic + ol] = W[ol, 128*ic + p]
        WT = const.tile([128, IC * O_SH], dt.bfloat16)

        eye = const.tile([128, 128], dt.bfloat16)
        nc.scalar.dma_start(eye[:, :], eye_d[:, :])
        ones = const.tile([1, 128], dt.bfloat16)
        nc.vector.memset(ones[:, :], 1.0)

        # bias broadcast to all 128 token partitions, once
        browf = const.tile([1, O_SH], dt.float32)
        nc.scalar.dma_start(browf[:, :], bias_d[:, :])
        brow = const.tile([1, O_SH], dt.bfloat16)
        nc.vector.tensor_copy(brow[:, :], browf[:, :])
        bias128 = const.tile([128, O_SH], dt.float32)
        for blk in range(O_SH // 512):
            pb = psb.tile([128, 512], dt.float32, tag="pb")
            nc.tensor.matmul(
                pb[:, :], ones[:, :], brow[:, 512 * blk : 512 * (blk + 1)],
                start=True, stop=True,
            )
            nc.scalar.copy(bias128[:, 512 * blk : 512 * (blk + 1)], pb[:, :])

        # ---- weight build: dequant scatter + CSR scatter-add ----
        Wts = []

        def build_tile(t):
            lutf = spool.tile([128, 16], dt.float32, tag="lutf")
            nc.scalar.dma_start(lutf[:, :], lut_d[t])
            lutb = spool.tile([128, 16], dt.bfloat16, tag="lutb")
            nc.vector.tensor_copy(lutb[:, :], lutf[:, :])
            # full-lut pattern repeated S times (log-doubling copies)
            pat = spool.tile([128, NI], dt.bfloat16, tag="pat")
            nc.vector.tensor_copy(pat[:, 0:16], lutb[:, :])
            sz = 16
            while sz < NI:
                cp = min(sz, NI - sz)
                nc.vector.tensor_copy(pat[:, sz : sz + cp], pat[:, 0:cp])
                sz += cp
            # csr tables
            ctb = spool.tile([128, NCH * CSE], dt.int16, tag="ctb")
            nc.scalar.dma_start(ctb[:, :], ctb_d[t])
            cvf = spool.tile([128, NCH * CSE], dt.float32, tag="cvf")
            nc.scalar.dma_start(cvf[:, :], cva_d[t])
            cvb = spool.tile([128, NCH * CSE], dt.bfloat16, tag="cvb")
            nc.vector.tensor_copy(cvb[:, :], cvf[:, :])
            Wt = wpool.tile([128, I], dt.bfloat16,
                            tag="Wlo" if t < OT // 2 else "Whi")
            for ch in range(NCH):
                sl = slice(ch * CH, (ch + 1) * CH)
                qix = qpool.tile([128, NI], dt.int16, tag="qix")
                nc.scalar.dma_start(qix[:, :], qid_d[t][:, ch * NI : (ch + 1) * NI])
                Wtm = mpool.tile([128, CH], dt.bfloat16, tag="Wtm")
                nc.gpsimd.local_scatter(
                    Wtm[:, :], pat[:, :], qix[:, :],
                    channels=128, num_elems=CH, num_idxs=NI,
                )
                Ct = mpool.tile([128, CH], dt.bfloat16, tag="Ct")
                nc.gpsimd.local_scatter(
                    Ct[:, :], cvb[:, ch * CSE : (ch + 1) * CSE],
                    ctb[:, ch * CSE : (ch + 1) * CSE],
                    channels=128, num_elems=CH, num_idxs=CSE,
                )
                nc.vector.tensor_add(Wt[:, sl], Wtm[:, :], Ct[:, :])
            return Wt

        def transpose_tile(t, Wt):
            for ic in range(IC):
                pt = pst.tile([128, 128], dt.bfloat16, tag="pt")
                nc.tensor.transpose(
                    pt[:, :], Wt[:, 128 * ic : 128 * (ic + 1)], eye[:, :]
                )
                nc.scalar.copy(
                    WT[:, O_SH * ic + 128 * t : O_SH * ic + 128 * (t + 1)],
                    pt[:, :],
                )

        # tiles 0-3 fully ready before phase A; tiles 4-7 scatter in the
        # background, their transposes interleave into phase A's PE stream
        for t in range(OT // 2):
            Wt = build_tile(t)
            transpose_tile(t, Wt)
        for t in range(OT // 2, OT):
            Wts.append(build_tile(t))

        # ---- GEMM in two o-phases; x streamed once per phase ----
        # late-tile transposes fire after these phase-A chunks
        tr_at = {6: 4, 11: 5, 16: 6, 21: 7}
        for blk in range(2):
            for g in range(NG):
                xg = []
                for ic in range(IC):
                    xf = xfpool.tile([128, G], dt.float32, tag="xf")
                    nc.sync.dma_start(xf[:, :], xt_d[ic][:, G * g : G * (g + 1)])
                    xb = xgpool.tile([128, G], dt.bfloat16, tag=f"xg{ic}")
                    nc.scalar.copy(xb[:, :], xf[:, :])
                    xg.append(xb)
                for nloc in range(CPG):
                    n = g * CPG + nloc
                    ps = psum.tile([128, 512], dt.float32, tag="ps")
                    for ic in range(IC):
                        nc.tensor.matmul(
                            ps[:, :],
                            xg[ic][:, 128 * nloc : 128 * (nloc + 1)],
                            WT[:, O_SH * ic + 512 * blk : O_SH * ic + 512 * (blk + 1)],
                            start=(ic == 0), stop=(ic == IC - 1),
                        )
                    yo = yopool.tile([128, 512], dt.float32, tag="yo")
                    nc.vector.tensor_add(
                        yo[:, :], ps[:, :],
                        bias128[:, 512 * blk : 512 * (blk + 1)],
                    )
                    nc.sync.dma_start(y_d[n][:, 512 * blk : 512 * (blk + 1)], yo[:, :])
                    if blk == 0 and n in tr_at:
                        transpose_tile(tr_at[n], Wts[tr_at[n] - OT // 2])

    nc.compile()
    return nc


def _prep_inputs(x, qweight, lut, rows, cols, vals, bias):
    x = np.ascontiguousarray(np.asarray(x, dtype=np.float32))
    qweight = np.asarray(qweight, dtype=np.int32)
    lut = np.asarray(lut, dtype=np.float32)
    rows = np.asarray(rows, dtype=np.int64)
    cols = np.asarray(cols, dtype=np.int64)
    vals = np.asarray(vals, dtype=np.float32)
    bias = np.asarray(bias, dtype=np.float32)

    idx = _host_indices(qweight)
    tbl, ctb, cvals, NI, CSE = _scatter_tables(idx, rows, cols, vals)

    x2 = x.reshape(NT_TOTAL, I)
    # i-major x per token shard (pure layout), shared by the 4 o-shard cores
    xts = []
    for nh in range(N_WAY):
        nsl = slice(N_SH * nh, N_SH * (nh + 1))
        xts.append(np.ascontiguousarray(x2[nsl].T).reshape(IC, 128, N_SH))
    in_maps = []
    for c in range(N_CORES):
        oq, nh = c % O_WAY, c // O_WAY
        osl = slice(O_SH * oq, O_SH * (oq + 1))
        in_maps.append(
            {
                "xt": xts[nh],
                "lut": np.ascontiguousarray(lut[osl].reshape(OT, 128, 16)),
                # chunk-major per o-tile row: [OT, 128, NCH*NI]
                "qidx": np.ascontiguousarray(tbl[osl].reshape(OT, 128, NCH * NI)),
                "ctb": np.ascontiguousarray(ctb[osl].reshape(OT, 128, NCH * CSE)),
                "cvals": np.ascontiguousarray(
                    cvals[osl].reshape(OT, 128, NCH * CSE)
                ),
                "bias": np.ascontiguousarray(bias[osl].reshape(1, O_SH)),
                "eye": _EYE,
            }
        )
    return in_maps, NI, CSE


def _run(inputs, trace=False, trace_kwargs=None):
    from concourse.bass_utils import run_bass_kernel_spmd

    in_maps, NI, CS = _prep_inputs(**inputs)

    key = (NI, CS)
    if key not in _GRAPH_CACHE:
        _GRAPH_CACHE[key] = _build_graph(NI, CS)
    nc = _GRAPH_CACHE[key]

    res = run_bass_kernel_spmd(
        nc, in_maps, core_ids=list(range(N_CORES)),
        trace=trace, **(trace_kwargs or {}),
    )
    out = np.empty((NT_TOTAL, O), np.float32)
    for c in range(N_CORES):
        oq, nh = c % O_WAY, c // O_WAY
        yc = res.results[c]["y"].reshape(N_SH, O_SH)
        out[N_SH * nh : N_SH * (nh + 1), O_SH * oq : O_SH * (oq + 1)] = yc
    return out.reshape(B, S, O), res


def kernel(x, qweight, lut, rows, cols, vals, bias):
    out, _ = _run(dict(x=x, qweight=qweight, lut=lut, rows=rows,
                       cols=cols, vals=vals, bias=bias))
    return out
